# revision 42
# baseline (speedup 1.0000x reference)
"""BlockDiagonalGRU Trainium2 kernel.

Math (per batch row b, per block n of 8, BLK=256):
  gates[b, n, :] = x[b, n*256:(n+1)*256] @ w_ih[n].T + h[b, ...] @ w_hh[n].T + b_ih[n] + b_hh[n]
  r = sigmoid(gates[..., 0:256]); u = sigmoid(gates[..., 256:512])
  c = tanh(r * gates[..., 512:768])
  h_new = (1-u)*h_blk + u*c

Sharding: data-parallel over batch across 8 cores (2048 rows each), weights
replicated (pre-transposed/cast to bf16 on host).

Per core, software-pipelined over 16 batch tiles of 128 rows:
  - SWDGE cast-load x,h fp32->bf16 natural [128b, 2048f], prefetch distance 2
  - PE transpose (bf16) feat chunks into [128 f_lo, 16 f_hi, 128 b]; the
    transposes for tile bt+1 are interleaved between tile bt's matmul groups
    so their LDWEIGHTS hide under matmul streaming; DVE evacuates PSUM->SBUF
  - PE: gates[batch, gate] += xT_chunk.T @ wT_chunk accumulated in PSUM.
    ps_ru [128, nn(2) x 512] bank-aligned N=512 matmuls; ps_c [128, nn x 256]
  - ACT: sigmoid (PSUM->SBUF bf16); DVE: r*g_c; ACT: tanh
  - blend in bf16 on DVE (final add emits fp32), pipelined one tile behind;
    last two tiles blend per-group to shorten the tail; HWDGE store.
"""

import numpy as np
import ml_dtypes

NUM_BLOCKS = 8
BLK = 256
D = 2048
B = 16384
N_CORES = 8
B_LOC = B // N_CORES  # 2048
P = 128
NBT = B_LOC // P  # 16 batch tiles per core
KC = 2  # k-chunks of 128 per block (256 feat)
G3 = 3 * BLK  # 768
BPG = 2  # blocks per PSUM group
NGRP = NUM_BLOCKS // BPG  # 4
NT = D // P  # 16 feat chunks
TPB = 4  # transposed chunks per PSUM-bank group

_nc_cache = {}


def _build(has_bias, reps=1):
    import concourse.mybir as mybir
    import concourse.tile as tile
    from concourse import bacc
    from concourse.masks import make_identity

    f32 = mybir.dt.float32
    bf16 = mybir.dt.bfloat16
    Sig = mybir.ActivationFunctionType.Sigmoid
    Tanh = mybir.ActivationFunctionType.Tanh

    nc = bacc.Bacc(None, target_bir_lowering=False)

    x_d = nc.dram_tensor("x", [B_LOC, D], f32, kind="ExternalInput")
    h_d = nc.dram_tensor("h", [B_LOC, D], f32, kind="ExternalInput")
    wt_d = nc.dram_tensor("wt", [P, 2 * NUM_BLOCKS * KC * G3], bf16, kind="ExternalInput")
    if has_bias:
        bias_d = nc.dram_tensor("bias", [P, NUM_BLOCKS * G3], f32, kind="ExternalInput")
    out_d = nc.dram_tensor("out", [B_LOC, D], f32, kind="ExternalOutput")
    warm_d = nc.dram_tensor("warm_scratch", [P, P], mybir.dt.bfloat16)

    with tile.TileContext(nc) as tc:
        with (
            tc.tile_pool(name="const", bufs=1) as cpool,
            tc.tile_pool(name="io", bufs=3) as io,
            tc.tile_pool(name="work", bufs=2) as work,
            tc.tile_pool(name="psru", bufs=2, space="PSUM") as psru_pool,
            tc.tile_pool(name="psc", bufs=2, space="PSUM") as psc_pool,
            tc.tile_pool(name="pst", bufs=2, space="PSUM") as pst_pool,
        ):
            ident = cpool.tile([P, P], bf16)
            make_identity(nc, ident)
            # weights: [128 k_lo, s(2), n(8), kc(2), 768 gates]
            wt = cpool.tile([P, 2, NUM_BLOCKS, KC, G3], bf16)
            if has_bias:
                bias_sb = cpool.tile([P, NUM_BLOCKS, 3, BLK], f32)

            def load_wt_chunk(s, nlo, nhi):
                base = (s * NUM_BLOCKS + nlo) * KC * G3
                end = (s * NUM_BLOCKS + nhi) * KC * G3
                # alternate HWDGE rings (sync/scalar) so the weight chunks get
                # a larger share of the SDMA engines' ring round-robin during
                # the startup bandwidth crunch
                eng = nc.sync if s == 0 else nc.scalar
                eng.dma_start(wt[:, s, nlo:nhi, :, :], wt_d[:, base:end])

            def load_tile(bt):
                row0 = bt * P
                x_nat = io.tile([P, D], bf16, tag="x_nat", name="x_nat")
                nc.gpsimd.dma_start(x_nat[:], x_d[row0 : row0 + P, :])
                h_nat = io.tile([P, D], bf16, tag="h_nat", name="h_nat", bufs=4)
                nc.gpsimd.dma_start(h_nat[:], h_d[row0 : row0 + P, :])
                return x_nat, h_nat

            def transpose_group(dst, src_nat, g):
                # transpose feat chunks [g*TPB, (g+1)*TPB) of src_nat into dst
                tp = pst_pool.tile([P, TPB * P], bf16, tag="pst", name="pst")
                for i in range(TPB):
                    t = g * TPB + i
                    nc.tensor.transpose(
                        tp[:, i * P : (i + 1) * P],
                        src_nat[:, t * P : (t + 1) * P],
                        ident[:],
                    )
                nc.vector.tensor_copy(dst[:, g * TPB : (g + 1) * TPB, :], tp[:])

            def gates_group(bt, grp, xt, ht, u_buf, c_buf):
                ps_ru = psru_pool.tile([P, BPG * 2 * BLK], f32, tag="psru", name="psru")
                ps_c = psc_pool.tile([P, BPG * BLK], f32, tag="psc", name="psc")
                for nn in range(BPG):
                    n = grp * BPG + nn
                    for s, tsrc in ((0, xt), (1, ht)):
                        for kc in range(KC):
                            lhsT = tsrc[:, KC * n + kc, :]
                            first = s == 0 and kc == 0
                            last = s == 1 and kc == KC - 1
                            nc.tensor.matmul(
                                ps_ru[:, nn * 2 * BLK : (nn + 1) * 2 * BLK],
                                lhsT,
                                wt[:, s, n, kc, 0 : 2 * BLK],
                                start=first,
                                stop=last,
                            )
                            nc.tensor.matmul(
                                ps_c[:, nn * BLK : (nn + 1) * BLK],
                                lhsT,
                                wt[:, s, n, kc, 2 * BLK : G3],
                                start=first,
                                stop=last,
                            )
                if has_bias:
                    for nn in range(BPG):
                        n = grp * BPG + nn
                        for g in range(2):
                            sl = slice(nn * 2 * BLK + g * BLK, nn * 2 * BLK + (g + 1) * BLK)
                            nc.vector.tensor_add(ps_ru[:, sl], ps_ru[:, sl], bias_sb[:, n, g, :])
                        nc.vector.tensor_add(
                            ps_c[:, nn * BLK : (nn + 1) * BLK],
                            ps_c[:, nn * BLK : (nn + 1) * BLK],
                            bias_sb[:, n, 2, :],
                        )
                col0 = grp * BPG * BLK
                col1 = (grp + 1) * BPG * BLK
                # one sigmoid over the whole [r|u] PSUM tile -> interleaved ru_buf
                ruc0 = grp * BPG * 2 * BLK
                ruc1 = (grp + 1) * BPG * 2 * BLK
                nc.scalar.activation(u_buf[:, ruc0:ruc1], ps_ru[:], Sig)
                r3 = u_buf[:, ruc0:ruc1].rearrange("p (a g b) -> p a g b", a=BPG, g=2)[
                    :, :, 0, :
                ]
                rc = work.tile([P, BPG * BLK], bf16, tag="rc", name="rc", bufs=3)
                nc.vector.tensor_mul(
                    rc[:].rearrange("p (a b) -> p a b", a=BPG),
                    r3,
                    ps_c[:].rearrange("p (a b) -> p a b", a=BPG),
                )
                nc.scalar.activation(c_buf[:, col0:col1], rc[:], Tanh)

            def u_view(u_buf, col0, col1):
                # u slices of the interleaved [r|u] buffer covering hidden
                # columns [col0, col1)
                nblk = (col1 - col0) // BLK
                return u_buf[:, 2 * col0 : 2 * col1].rearrange(
                    "p (a g b) -> p a g b", a=nblk, g=2
                )[:, :, 1, :]

            def blend_full(bt, h_nat, u_buf, c_buf):
                row0 = bt * P
                d_t = work.tile([P, D], bf16, tag="d_t", name="d_t")
                e_t = work.tile([P, D], bf16, tag="e_t", name="e_t")
                nc.vector.tensor_sub(d_t[:], c_buf[:], h_nat[:])
                nc.vector.tensor_mul(
                    e_t[:].rearrange("p (a b) -> p a b", b=BLK),
                    u_view(u_buf, 0, D),
                    d_t[:].rearrange("p (a b) -> p a b", b=BLK),
                )
                hnew = work.tile([P, D], f32, tag="hnew", name="hnew")
                nc.vector.tensor_add(hnew[:], h_nat[:], e_t[:])
                nc.sync.dma_start(out_d[row0 : row0 + P, :], hnew[:])

            def blend_grp(bt, grp, h_nat, u_buf, c_buf):
                row0 = bt * P
                col0 = grp * BPG * BLK
                col1 = (grp + 1) * BPG * BLK
                d_t = work.tile([P, BPG * BLK], bf16, tag="d_g", name="d_g")
                e_t = work.tile([P, BPG * BLK], bf16, tag="e_g", name="e_g")
                nc.vector.tensor_sub(d_t[:], c_buf[:, col0:col1], h_nat[:, col0:col1])
                nc.vector.tensor_mul(
                    e_t[:].rearrange("p (a b) -> p a b", b=BLK),
                    u_view(u_buf, col0, col1),
                    d_t[:].rearrange("p (a b) -> p a b", b=BLK),
                )
                hnew = work.tile([P, BPG * BLK], f32, tag="hnew_g", name="hnew_g")
                nc.vector.tensor_add(hnew[:], h_nat[:, col0:col1], e_t[:])
                nc.sync.dma_start(out_d[row0 : row0 + P, col0:col1], hnew[:])

            def warmup():
                # dummy matmuls while the first loads are in flight: keeps the
                # PE HAM activity monitor busy so real matmuls start at 2.4GHz
                ps = psc_pool.tile([P, BPG * BLK], f32, tag="psc", name="psc_warm")
                NWU = 100
                for i in range(NWU):
                    nc.tensor.matmul(
                        ps[:, 0:P],
                        ident[:],
                        ident[:],
                        start=(i == 0),
                        stop=(i == NWU - 1),
                    )
                sc = work.tile([P, P], bf16, tag="warm_sb", name="warm_sb", bufs=1)
                nc.vector.tensor_copy(sc[:], ps[:, 0:P])
                nc.scalar.dma_start(warm_d[:, :], sc[:])

            def load_tile0():
                # first tile loads split in halves so transposes start earlier
                x_nat = io.tile([P, D], bf16, tag="x_nat", name="x_nat")
                h_nat = io.tile([P, D], bf16, tag="h_nat", name="h_nat", bufs=4)
                HD = D // 2
                nc.gpsimd.dma_start(x_nat[:, 0:HD], x_d[0:P, 0:HD])
                nc.gpsimd.dma_start(h_nat[:, 0:HD], h_d[0:P, 0:HD])
                nc.gpsimd.dma_start(x_nat[:, HD:D], x_d[0:P, HD:D])
                nc.gpsimd.dma_start(h_nat[:, HD:D], h_d[0:P, HD:D])
                return x_nat, h_nat

            def body(_iv=None):
                warmup()
                nats = {0: load_tile0()}
                # startup DMAs ordered by first use
                load_wt_chunk(0, 0, 2)
                load_wt_chunk(1, 0, 2)
                load_wt_chunk(0, 2, 4)
                load_wt_chunk(1, 2, 4)
                nats[1] = load_tile(1)
                load_wt_chunk(0, 4, 6)
                load_wt_chunk(1, 4, 6)
                load_wt_chunk(0, 6, 8)
                load_wt_chunk(1, 6, 8)
                if has_bias:
                    nc.sync.dma_start(bias_sb[:, :, :, :], bias_d[:, :])
                xts = {}
                pending = None
                for bt in range(NBT):
                    if bt == 0:
                        xt = io.tile([P, NT, P], bf16, tag="xt", name="xt")
                        ht = io.tile([P, NT, P], bf16, tag="ht", name="ht")
                        x_nat, h_nat = nats.pop(0)
                    else:
                        x_nat, h_nat = nats.pop(bt)
                        xt, ht = xts.pop(bt)
                    if bt + 1 < NBT:
                        xt1 = io.tile([P, NT, P], bf16, tag="xt", name="xt")
                        ht1 = io.tile([P, NT, P], bf16, tag="ht", name="ht")
                        xts[bt + 1] = (xt1, ht1)

                    # interleaved [r|u] sigmoid outputs: [128, n(8) x {r,u} x 256]
                    u_buf = work.tile([P, 2 * D], bf16, tag="u_buf", name="u_buf", bufs=3)
                    c_buf = work.tile([P, D], bf16, tag="c_buf", name="c_buf", bufs=3)

                    # schedule of next-tile (or, for bt 0, own) transpose groups
                    # across this tile's matmul groups
                    for grp in range(NGRP):
                        if grp == 3 and bt + 2 < NBT:
                            # prefetch two tiles ahead, late in the loop so the
                            # startup weight DMAs win the early SDMA bandwidth
                            nats[bt + 2] = load_tile(bt + 2)
                        if bt == 0:
                            # transpose own chunks just ahead of their matmuls
                            g = grp * BPG * KC // TPB
                            if grp * BPG * KC % TPB == 0:
                                transpose_group(xt, x_nat, g)
                                transpose_group(ht, h_nat, g)
                        gates_group(bt, grp, xt, ht, u_buf, c_buf)
                        if bt + 1 < NBT:
                            # interleave next tile's transposes between matmul
                            # groups: their LDWEIGHTS hide under MM streaming.
                            # For bt 0, defer them to groups 2/3 so the just-
                            # issued x1/h1 loads have time to land.
                            xn1, hn1 = nats[bt + 1]
                            if bt == 0:
                                if grp == 2:
                                    for g in range(NT // TPB):
                                        transpose_group(xts[1][0], xn1, g)
                                elif grp == 3:
                                    for g in range(NT // TPB):
                                        transpose_group(xts[1][1], hn1, g)
                            else:
                                transpose_group(xts[bt + 1][0], xn1, grp)
                                transpose_group(xts[bt + 1][1], hn1, grp)
                        if bt >= NBT - 2:
                            blend_grp(bt, grp, h_nat, u_buf, c_buf)
                    if pending is not None:
                        blend_full(*pending)
                        pending = None
                    if bt < NBT - 2:
                        pending = (bt, h_nat, u_buf, c_buf)

            if reps == 1:
                body()
            else:
                with tc.For_i(0, reps, 1) as iv:
                    body(iv)

    nc.compile()
    return nc


def _get_nc(has_bias, reps=1):
    key = (has_bias, reps)
    if key not in _nc_cache:
        _nc_cache[key] = _build(has_bias, reps)
    return _nc_cache[key]


def _prep_weights(w_ih, w_hh):
    # want wt[p, s, n, kc, g] = W_s[n, g, kc*128 + p]
    def tr(w):
        return np.ascontiguousarray(
            w.reshape(NUM_BLOCKS, G3, KC, P).transpose(3, 0, 2, 1)
        )  # [128, n, kc, g]

    wt = np.stack([tr(w_ih), tr(w_hh)], axis=1)  # [128, s, n, kc, g]
    return np.ascontiguousarray(wt.reshape(P, -1).astype(ml_dtypes.bfloat16))


def kernel(x, h, w_ih, w_hh, b_ih, b_hh, _reps=1, _nc=None):
    from concourse.bass_utils import run_bass_kernel_spmd

    x = np.asarray(x, dtype=np.float32)
    h = np.asarray(h, dtype=np.float32)
    w_ih = np.asarray(w_ih, dtype=np.float32)
    w_hh = np.asarray(w_hh, dtype=np.float32)
    bsum = np.asarray(b_ih, dtype=np.float32) + np.asarray(b_hh, dtype=np.float32)
    has_bias = bool(np.any(bsum))

    wt = _prep_weights(w_ih, w_hh)
    nc = _nc if _nc is not None else _get_nc(has_bias, _reps)

    in_maps = []
    for c in range(N_CORES):
        m = {
            "x": np.ascontiguousarray(x[c * B_LOC : (c + 1) * B_LOC]),
            "h": np.ascontiguousarray(h[c * B_LOC : (c + 1) * B_LOC]),
            "wt": wt,
        }
        if has_bias:
            brep = np.broadcast_to(
                bsum.reshape(1, NUM_BLOCKS * G3), (P, NUM_BLOCKS * G3)
            )
            m["bias"] = np.ascontiguousarray(brep)
        in_maps.append(m)

    res = run_bass_kernel_spmd(nc, in_maps, core_ids=list(range(N_CORES)))
    out = np.concatenate([res.results[c]["out"] for c in range(N_CORES)], axis=0)
    return np.ascontiguousarray(out.astype(np.float32))
